# revision 44
# baseline (speedup 1.0000x reference)
"""Trainium2 Bass kernel for nn_DownBlock_res_dct1 (maxpool 2x2 + truncated
block-DCT low-pass + SE attention + 1x1 conv + two 3x3 convs), data-parallel
over the batch across 8 NeuronCores.

Self-contained: hardcodes all shapes/constants; builds one SPMD Bass module
(one batch item per core), runs via run_bass_kernel_spmd, gathers the full
(8, 128, 192, 192) output.

Per-core layout: partitions p = half*64 + ch, where half selects image row
halves.  Key schedule points: SE stats are computed directly from the 4 DCT
coefficients (Parseval-style with gather multiplicities), so gamma is ready
right after the HBM-bound input load instead of after full y1 expansion;
the y1 expansion/gather and the att 1x1 conv overlap the conv phase.  All
matmuls run as full 128x128 PE tiles (zero-padded weights where K=64), and
conv1 pairs taps (0,dx)+(1,dx) into single K=128 matmuls via a rolling
row-shifted duplicate of x_all (6 matmuls per row-pair instead of 9).
Matmuls in bf16 with fp32 PSUM accumulation; cross-half SE reduction via a
tiny fp32 matmul against a stacked identity.
"""

import math
from contextlib import ExitStack

import numpy as np

import concourse.bass as bass
import concourse.mybir as mybir
import concourse.tile as tile
from concourse import bacc
from concourse.bass_utils import run_bass_kernel_spmd

FP32 = mybir.dt.float32
BF16 = mybir.dt.bfloat16
AX = mybir.AxisListType
OP = mybir.AluOpType
ACT = mybir.ActivationFunctionType

N = 8  # DCT block size
_P8 = np.arange(8)
COS1 = np.cos(math.pi * (_P8 + 0.5) / 8.0 * 1).astype(np.float64)
COS2 = np.cos(math.pi * (_P8 + 0.5) / 8.0 * 2).astype(np.float64)
# Selected zigzag coeffs [0,1,2,5] -> (k1,k2) in {(0,0),(0,1),(1,0),(0,2)}
A00 = (1.0 / 8.0) ** 2
A01 = 2.0 / 64.0
A02 = 2.0 / 64.0
A10 = 2.0 / 64.0


def _runs(idx):
    """Contiguous runs where idx[i] = i - g: list of (out_start, in_start, len)."""
    runs = []
    s = 0
    for i in range(1, len(idx) + 1):
        if i == len(idx) or idx[i] != idx[i - 1] + 1:
            runs.append((s, int(idx[s]), i - s))
            s = i
    return runs


def _split_at(ro, rin, rl, bound):
    """Split a run at source-row `bound`."""
    if rin < bound < rin + rl:
        return [(ro, rin, bound - rin), (ro + bound - rin, bound, rin + rl - bound)]
    return [(ro, rin, rl)]


def _stats_consts(h, T):
    """Per-partition row weights + shared col weights for computing
    sum(y1)/sum(y1^2) directly from the 4 stored DCT coefficients.

    Stored coeffs (per block): c00 raw, c01 = A01*<cos1,a0>, c02 =
    A02*<cos2,a0>, c10 raw; recon = A00*c00 + c01*cos1[q] + c02*cos2[q]
    + A10*c10*cos1[r].  y1 = recon[hi][:,wi] with duplicate rows/cols.
    Returns statc [128, 14, 36]: [:, k, 0:12] row weight (local block i,
    half-dependent), [:, k, 12:36] col weight.  k=0..9: S2 pairs in
    _STAT_PAIRS order; k=10..13: S1 terms (00, 01, 02, 10).
    """
    hi = (np.arange(h) * (h - (N - 1))) // h
    m = np.zeros(h, np.float64)
    for u in range(h):
        m[hi[u]] += 1.0
    cos1 = COS1
    cos2 = COS2
    ones = np.ones(8)

    def wsum(f):
        out = np.zeros(2 * T)
        for i in range(2 * T):
            for r in range(8):
                out[i] += m[8 * i + r] * f[r]
        return out

    RF1, RFc1, R2c1c1 = wsum(ones), wsum(cos1), wsum(cos1 * cos1)
    CG1, CGc1, CGc2 = RF1, RFc1, wsum(cos2)
    C2c1c1, C2c1c2, C2c2c2 = R2c1c1, wsum(cos1 * cos2), wsum(cos2 * cos2)

    lam = {"00": A00, "01": 1.0, "02": 1.0, "10": A10}
    fkey = {"00": "1", "01": "1", "02": "1", "10": "c1"}
    gkey = {"00": "1", "01": "c1", "02": "c2", "10": "1"}
    Rmap = {("1", "1"): RF1, ("1", "c1"): RFc1, ("c1", "c1"): R2c1c1}
    Cmap = {
        ("1", "1"): CG1, ("1", "c1"): CGc1, ("1", "c2"): CGc2,
        ("c1", "c1"): C2c1c1, ("c1", "c2"): C2c1c2, ("c2", "c2"): C2c2c2,
    }
    statc = np.zeros((128, 14, 36), np.float32)
    for k, (t, tp) in enumerate(_STAT_PAIRS):
        s = (1.0 if t == tp else 2.0) * lam[t] * lam[tp]
        rw = s * Rmap[tuple(sorted((fkey[t], fkey[tp])))]
        cw = Cmap[tuple(sorted((gkey[t], gkey[tp])))]
        for hf in (0, 1):
            statc[hf * 64 : (hf + 1) * 64, k, 0:T] = rw[hf * T : (hf + 1) * T]
        statc[:, k, 12 : 12 + 2 * T] = cw
    CGmap = {"1": CG1, "c1": CGc1, "c2": CGc2}
    for k, t in enumerate(["00", "01", "02", "10"]):
        rw = lam[t] * Rmap[tuple(sorted((fkey[t], "1")))]
        for hf in (0, 1):
            statc[hf * 64 : (hf + 1) * 64, 10 + k, 0:T] = rw[hf * T : (hf + 1) * T]
        statc[:, 10 + k, 12 : 12 + 2 * T] = CGmap[gkey[t]]
    return statc


_STAT_PAIRS = [
    ("00", "00"), ("00", "01"), ("00", "02"), ("00", "10"), ("01", "01"),
    ("01", "02"), ("01", "10"), ("02", "02"), ("02", "10"), ("10", "10"),
]


def build_nc(H=384, W=384, debug=False):
    C, C2 = 64, 128
    h, w = H // 2, W // 2
    hh = h // 2  # rows per half
    assert hh % N == 0 and w % N == 0
    T = hh // N  # block-rows per half
    S = w // N  # block-cols
    CH = 64

    hi = (np.arange(h) * (h - (N - 1))) // h
    wi = (np.arange(w) * (w - (N - 1))) // w
    col_runs = _runs(wi)
    row_runs_h = [_runs(hi[hh * hf : hh * (hf + 1)]) for hf in (0, 1)]

    MP_CH = 4  # pooled rows per maxpool chunk
    n_mp = hh // MP_CH
    DCT_T = 4 if T % 4 == 0 else (2 if T % 2 == 0 else T)
    n_dct = T // DCT_T
    XPC = DCT_T * N  # xp tile rows
    mp_per_xpc = XPC // MP_CH
    assert XPC % MP_CH == 0

    nc = bacc.Bacc("TRN2")

    x = nc.dram_tensor("x", [C, H, W], FP32, kind="ExternalInput")
    w1 = nc.dram_tensor("w1", [C2, C, 3, 3], FP32, kind="ExternalInput")
    b1 = nc.dram_tensor("b1", [C2], FP32, kind="ExternalInput")
    w2 = nc.dram_tensor("w2", [C2, C2, 3, 3], FP32, kind="ExternalInput")
    b2 = nc.dram_tensor("b2", [C2], FP32, kind="ExternalInput")
    attw = nc.dram_tensor("att_conv_w", [C, C, 1, 1], FP32, kind="ExternalInput")
    attb = nc.dram_tensor("att_conv_b", [C], FP32, kind="ExternalInput")
    fc1 = nc.dram_tensor("fc1_w", [C // 16, C, 1, 1], FP32, kind="ExternalInput")
    fc2 = nc.dram_tensor("fc2_w", [C, C // 16, 1, 1], FP32, kind="ExternalInput")
    out = nc.dram_tensor("out", [C2, h, w], FP32, kind="ExternalOutput")

    dbg = {}
    if debug:
        for name, shape in [
            ("dbg_xp", [128, hh, w]),
            ("dbg_y1", [128, hh, w]),
            ("dbg_se", [64, 8]),
            ("dbg_gamma", [64, 1]),
            ("dbg_xall", [128, hh + 2, w + 2]),
            ("dbg_o1", [C2, h + 2, w + 2]),
        ]:
            dbg[name] = nc.dram_tensor(name, shape, FP32, kind="ExternalOutput")

    const_np = np.zeros((128, 4, 8), np.float32)
    const_np[:, 0, :] = COS1
    const_np[:, 1, :] = COS2
    const_np[:, 2, :] = COS1 * A01
    const_np[:, 3, :] = COS2 * A02
    cdram = nc.inline_tensor(const_np.reshape(128, 32), name="dctconst")
    statc_np = _stats_consts(h, T)
    statdram = nc.inline_tensor(
        np.ascontiguousarray(statc_np.reshape(128, 14 * 36)), name="statconst"
    )

    NF = h * w  # pixels per full channel image

    with tile.TileContext(nc) as tc, ExitStack() as ctx:
        wpool = ctx.enter_context(tc.tile_pool(name="wpool", bufs=1))
        if debug:
            dpool = ctx.enter_context(tc.tile_pool(name="dpool", bufs=1))
        small = ctx.enter_context(tc.tile_pool(name="small", bufs=2))
        smallD = tc.alloc_tile_pool(name="smallD", bufs=1)
        psA = ctx.enter_context(tc.tile_pool(name="psA", bufs=3, space="PSUM"))
        psC = ctx.enter_context(tc.tile_pool(name="psC", bufs=4, space="PSUM"))
        # phase-scoped pools; each SBUF side is a LIFO stack.
        # left:  ... pxp prec | pop prec, pop pxp, push pxa, push po1
        # right: py1 pin      | pop pin (after maxpool), pop py1 (at end)
        py1 = tc.alloc_tile_pool(name="py1", bufs=1, side="right")
        pxp = tc.alloc_tile_pool(name="pxp", bufs=1, side="right")
        pin = tc.alloc_tile_pool(name="pin", bufs=3, side="right")

        # ---------------- constants / weights ----------------
        consts = wpool.tile([128, 4, 8], FP32)
        nc.sync.dma_start(consts[:], cdram[:].rearrange("p (a b) -> p a b", a=4))
        statc = wpool.tile([128, 14, 36], FP32)
        nc.sync.dma_start(statc[:], statdram[:].rearrange("p (a b) -> p a b", a=14))
        # persistent DCT coefficients: [:, t, i, j] for t in (00, 01, 02, 10)
        ctile = wpool.tile([128, 4, T, S], FP32)

        def cvec(row, shp):  # broadcast [128,8] const row to shp (q innermost)
            return consts[:, row, None, None, :].to_broadcast(shp)

        from concourse.masks import make_identity

        ident = wpool.tile([128, 128], FP32)
        make_identity(nc, ident[:])
        combT = wpool.tile([128, 64], FP32)
        nc.vector.tensor_copy(combT[0:CH, :], ident[0:CH, 0:CH])
        nc.vector.tensor_copy(combT[CH:128, :], ident[0:CH, 0:CH])

        zerot = wpool.tile([128, 1], FP32)
        nc.vector.memset(zerot[:], 0.0)
        warmt = wpool.tile([128, 2], FP32)
        nc.scalar.activation(warmt[:, 0:1], zerot[:], ACT.Sigmoid)
        nc.scalar.activation(warmt[:, 1:2], zerot[:], ACT.Relu)

        w1s = wpool.tile([C2, C * 9], FP32)
        nc.sync.dma_start(w1s[:], w1[:].rearrange("o i ky kx -> o (i ky kx)"))
        # conv1 runs on a row-shifted duplicate tile (partitions 64..127 hold
        # the next image row), so taps (0,dx),(1,dx) pair into one K=128
        # matmul; taps (2,dx) are zero-padded singles on the shifted half.
        w1pair = wpool.tile([128, 3, C2], BF16)
        w1sing = wpool.tile([128, 3, C2], BF16)
        nc.vector.memset(w1sing[0:CH, :, :], 0.0)
        for tap in range(9):
            dy, dx = divmod(tap, 3)
            pt = psA.tile([C, C2], FP32, tag="ps")
            sv = w1s[:].rearrange("o (i t) -> o t i", t=9)[:, tap, :]
            nc.tensor.transpose(pt[:], sv, ident[:])
            if dy == 0:
                nc.vector.tensor_copy(w1pair[0:CH, dx, :], pt[:])
            elif dy == 1:
                nc.vector.tensor_copy(w1pair[CH:128, dx, :], pt[:])
            else:
                nc.vector.tensor_copy(w1sing[CH:128, dx, :], pt[:])

        w2s = wpool.tile([C2, C2 * 9], FP32)
        nc.sync.dma_start(w2s[:], w2[:].rearrange("o i ky kx -> o (i ky kx)"))
        w2t = wpool.tile([128, 9, C2], BF16)
        for tap in range(9):
            pt = psA.tile([C2, C2], FP32, tag="ps")
            sv = w2s[:].rearrange("o (i t) -> o t i", t=9)[:, tap, :]
            nc.tensor.transpose(pt[:], sv, ident[:])
            nc.vector.tensor_copy(w2t[:, tap, :], pt[:])

        atts = wpool.tile([C, C], FP32)
        nc.sync.dma_start(atts[:], attw[:, :, 0, 0])
        attt = wpool.tile([128, C], FP32)
        pt = psA.tile([C, C], FP32, tag="ps")
        nc.tensor.transpose(pt[:], atts[:], ident[0:C, 0:C])
        nc.vector.tensor_copy(attt[0:CH, :], pt[:])
        nc.vector.tensor_copy(attt[CH:128, :], pt[:])

        fc1t = wpool.tile([C, C // 16], FP32)
        nc.sync.dma_start(fc1t[:], fc1[:, :, 0, 0].rearrange("o c -> c o"))
        fc1b = wpool.tile([C, C // 16], BF16)
        nc.vector.tensor_copy(fc1b[:], fc1t[:])
        fc2t = wpool.tile([C // 16, C], FP32)
        nc.sync.dma_start(fc2t[:], fc2[:, :, 0, 0].rearrange("o c -> c o"))
        fc2b = wpool.tile([C // 16, C], BF16)
        nc.vector.tensor_copy(fc2b[:], fc2t[:])

        b1t = wpool.tile([C2, 1], FP32)
        nc.sync.dma_start(b1t[:], b1[:, None])
        b2t = wpool.tile([C2, 1], FP32)
        nc.sync.dma_start(b2t[:], b2[:, None])
        attbt = wpool.tile([C, 1], FP32)
        nc.sync.dma_start(attbt[:], attb[:, None])


        # ---------------- load + maxpool ----------------
        xp_tiles = [
            pxp.tile([128, XPC, w], BF16, tag=f"xp{i}", name=f"xp{i}")
            for i in range(n_dct)
        ]
        xdf = x[:].rearrange("c r q -> c (r q)")
        for k in range(n_mp):
            xin = pin.tile([128, 2 * MP_CH, W], FP32, tag="xin")
            r0 = 2 * MP_CH * k
            xin_f = xin[:].rearrange("p r q -> p (r q)")
            nc.sync.dma_start(
                xin_f[0:CH, :], xdf[:, r0 * W : (r0 + 2 * MP_CH) * W]
            )
            nc.sync.dma_start(
                xin_f[CH:128, :],
                xdf[:, (H // 2 + r0) * W : (H // 2 + r0 + 2 * MP_CH) * W],
            )
            hmax = pin.tile([128, 2 * MP_CH, w], BF16, tag="hmax")
            xv = xin[:].rearrange("p r (a two) -> p r a two", two=2)
            nc.vector.tensor_tensor(hmax[:], xv[:, :, :, 0], xv[:, :, :, 1], OP.max)
            xpt = xp_tiles[k // mp_per_xpc]
            rr = (k % mp_per_xpc) * MP_CH
            hv = hmax[:].rearrange("p (b two) q -> p b two q", two=2)
            nc.vector.tensor_tensor(
                xpt[:, rr : rr + MP_CH, :], hv[:, :, 0, :], hv[:, :, 1, :], OP.max
            )

        pin.release()

        # ---------------- DCT coefficients + expansion + gather ----------------
        # gather pieces keyed by the recon (source) strip that completes them
        pieces_by_strip = [[] for _ in range(n_dct)]
        for hf in (0, 1):
            for ro, rin_g, rl in row_runs_h[hf]:
                for ro2, rin2, rl2 in _split_at(ro, rin_g, rl, hh):
                    src_hf = 0 if rin2 < hh else 1
                    rin_l = rin2 - hh * src_hf
                    st = rin_l
                    while st < rin_l + rl2:
                        en = min(rin_l + rl2, (st // XPC + 1) * XPC)
                        pieces_by_strip[st // XPC].append(
                            (hf, ro2 + (st - rin_l), src_hf, st, en - st)
                        )
                        st = en

        prec = tc.alloc_tile_pool(name="prec", bufs=1)
        recon = prec.tile([128, hh, w], BF16)  # pre-gather reconstruction
        y1 = py1.tile([128, hh, w], BF16)
        shp4 = (128, DCT_T, S, N)
        def exp_gather(c):
            csl = slice(c * DCT_T, (c + 1) * DCT_T)
            e0 = smallD.tile([128, DCT_T, w], BF16, tag="e0")
            e0v = e0[:].rearrange("p t (s q) -> p t s q", q=N)
            tmp8 = smallD.tile([128, DCT_T, w], BF16, tag="tmp8")
            tmp8v = tmp8[:].rearrange("p t (s q) -> p t s q", q=N)
            c01b = ctile[:, 1, csl, :, None].to_broadcast(shp4)
            c02b = ctile[:, 2, csl, :, None].to_broadcast(shp4)
            c00b = ctile[:, 0, csl, :, None].to_broadcast(shp4)
            nc.vector.tensor_tensor(e0v, c01b, cvec(0, shp4), OP.mult)
            nc.vector.tensor_tensor(tmp8v, c02b, cvec(1, shp4), OP.mult)
            nc.vector.tensor_tensor(e0[:], e0[:], tmp8[:], OP.add)
            nc.vector.scalar_tensor_tensor(e0v, c00b, A00, e0v, OP.mult, OP.add)

            c10e = smallD.tile([128, DCT_T, w], BF16, tag="c10e")
            c10ev = c10e[:].rearrange("p t (s q) -> p t s q", q=N)
            nc.scalar.copy(c10ev, ctile[:, 3, csl, :, None].to_broadcast(shp4))

            rv = recon[:, c * XPC : (c + 1) * XPC, :].rearrange(
                "p (t r) q -> p t r q", r=N
            )
            for r in range(N):
                nc.vector.scalar_tensor_tensor(
                    rv[:, :, r, :], c10e[:], float(A10 * COS1[r]), e0[:],
                    OP.mult, OP.add,
                )

            for hf, ro2, src_hf, rin_l, rl2 in pieces_by_strip[c]:
                pb = hf * CH
                pbi = src_hf * CH
                if pbi != pb:
                    # cross-half rows: bounce through DMA into a
                    # base-aligned staging tile
                    xstage = small.tile([128, N, w], BF16, tag="xstage")
                    nc.sync.dma_start(
                        xstage[pb : pb + CH, 0:rl2, :],
                        recon[pbi : pbi + CH, rin_l : rin_l + rl2, :],
                    )
                    srct, srow, spb = xstage, 0, pb
                else:
                    srct, srow, spb = recon, rin_l, pbi
                for ci, (co, cin, cl) in enumerate(col_runs):
                    sv = srct[spb : spb + CH, srow : srow + rl2, cin : cin + cl]
                    dv = y1[pb : pb + CH, ro2 : ro2 + rl2, co : co + cl]
                    if (ci + hf) % 2 == 0:
                        nc.vector.tensor_copy(dv, sv)
                    else:
                        nc.scalar.copy(dv, sv)

        for c in range(n_dct):
            csl = slice(c * DCT_T, (c + 1) * DCT_T)
            xpt = xp_tiles[c]
            a0 = smallD.tile([128, DCT_T, w], FP32, tag="a0")
            xv_row = xpt[:].rearrange("p (t r) q -> p t r q", r=N)
            a0b = smallD.tile([128, DCT_T, w], FP32, tag="a0b")
            nc.vector.tensor_tensor(
                a0[:], xv_row[:, :, 0, :], xv_row[:, :, 1, :], OP.add
            )
            nc.vector.tensor_tensor(
                a0b[:], xv_row[:, :, 2, :], xv_row[:, :, 3, :], OP.add
            )
            nc.vector.tensor_tensor(a0[:], a0[:], a0b[:], OP.add)
            nc.vector.tensor_tensor(
                a0b[:], xv_row[:, :, 4, :], xv_row[:, :, 5, :], OP.add
            )
            nc.vector.tensor_tensor(a0[:], a0[:], a0b[:], OP.add)
            nc.vector.tensor_tensor(
                a0b[:], xv_row[:, :, 6, :], xv_row[:, :, 7, :], OP.add
            )
            nc.vector.tensor_tensor(a0[:], a0[:], a0b[:], OP.add)
            a1 = smallD.tile([128, DCT_T, w], FP32, tag="a1")
            for r in range(N):
                if r == 0:
                    nc.vector.tensor_scalar(
                        a1[:], xv_row[:, :, r, :], float(COS1[0]), None, OP.mult
                    )
                else:
                    nc.vector.scalar_tensor_tensor(
                        a1[:], xv_row[:, :, r, :], float(COS1[r]), a1[:],
                        OP.mult, OP.add,
                    )
            a0v = a0[:].rearrange("p t (s q) -> p t s q", q=N)
            a1v = a1[:].rearrange("p t (s q) -> p t s q", q=N)
            nc.vector.tensor_reduce(ctile[:, 0, csl, :], a0v, axis=AX.X, op=OP.add)
            nc.vector.tensor_reduce(ctile[:, 3, csl, :], a1v, axis=AX.X, op=OP.add)
            tmp = smallD.tile([128, DCT_T, S, N], FP32, tag="ctmp")
            nc.vector.tensor_tensor(tmp[:], a0v, cvec(2, shp4), OP.mult)
            nc.vector.tensor_reduce(ctile[:, 1, csl, :], tmp[:], axis=AX.X, op=OP.add)
            nc.vector.tensor_tensor(tmp[:], a0v, cvec(3, shp4), OP.mult)
            nc.vector.tensor_reduce(ctile[:, 2, csl, :], tmp[:], axis=AX.X, op=OP.add)

            if c < n_dct - 1:
                exp_gather(c)

        # ---------------- SE stats from coefficients ----------------
        TIDX = {"00": 0, "01": 1, "02": 2, "10": 3}
        red1 = smallD.tile([128, 14, T], FP32, tag="red1")
        for b in range(5):
            ks = slice(2 * b, 2 * (b + 1))
            pbig = smallD.tile([128, 2, T, S], FP32, tag="pbig")
            for kk, (t, tp) in enumerate(_STAT_PAIRS[ks]):
                nc.vector.tensor_tensor(
                    pbig[:, kk, :, :], ctile[:, TIDX[t], :, :],
                    ctile[:, TIDX[tp], :, :], OP.mult,
                )
            nc.vector.tensor_tensor(
                pbig[:], pbig[:],
                statc[:, ks, None, 12 : 12 + S].to_broadcast((128, 2, T, S)),
                OP.mult,
            )
            nc.vector.tensor_reduce(red1[:, ks, :], pbig[:], axis=AX.X, op=OP.add)
        for b in range(2):
            ks = slice(10 + 2 * b, 10 + 2 * (b + 1))
            pbig = smallD.tile([128, 2, T, S], FP32, tag="pbig")
            nc.vector.tensor_tensor(
                pbig[:], ctile[:, 2 * b : 2 * (b + 1), :, :],
                statc[:, ks, None, 12 : 12 + S].to_broadcast((128, 2, T, S)),
                OP.mult,
            )
            nc.vector.tensor_reduce(red1[:, ks, :], pbig[:], axis=AX.X, op=OP.add)
        nc.vector.tensor_tensor(red1[:], red1[:], statc[:, :, 0:T], OP.mult)
        s12 = small.tile([128, 2], FP32, tag="s12")
        nc.vector.tensor_reduce(
            s12[:, 1:2], red1[:, 0:10, :].rearrange("p a b -> p (a b)"),
            axis=AX.X, op=OP.add,
        )
        nc.vector.tensor_reduce(
            s12[:, 0:1], red1[:, 10:14, :].rearrange("p a b -> p (a b)"),
            axis=AX.X, op=OP.add,
        )

        # ---------------- SE ----------------
        st = small.tile([64, 12], FP32, tag="se")
        pcomb = psA.tile([64, 2], FP32, tag="ps")
        nc.tensor.matmul(pcomb[:], combT[:], s12[:], start=True, stop=True)
        nc.vector.tensor_scalar(st[:, 2:3], pcomb[:, 0:1], 1.0 / NF, None, OP.mult)
        nc.vector.tensor_scalar(st[:, 3:4], pcomb[:, 1:2], 1.0 / NF, None, OP.mult)
        nc.vector.tensor_tensor(st[:, 4:5], st[:, 2:3], st[:, 2:3], OP.mult)
        nc.vector.tensor_tensor(st[:, 5:6], st[:, 3:4], st[:, 4:5], OP.subtract)
        nc.vector.tensor_scalar(
            st[:, 6:7], st[:, 5:6], float(NF) / float(NF - 1), None, OP.mult
        )
        nc.vector.tensor_tensor(st[:, 7:8], st[:, 2:3], st[:, 6:7], OP.add)
        sb = small.tile([64, 1], BF16, tag="sb16")
        nc.vector.tensor_copy(sb[:], st[:, 7:8])
        pfc1 = psA.tile([C // 16, 1], FP32, tag="ps")
        nc.tensor.matmul(pfc1[:], fc1b[:], sb[:], start=True, stop=True)
        tb = small.tile([C // 16, 1], BF16, tag="tb16")
        nc.scalar.activation(tb[:], pfc1[:], ACT.Relu)
        pfc2 = psA.tile([C, 1], FP32, tag="ps")
        nc.tensor.matmul(pfc2[:], fc2b[:], tb[:], start=True, stop=True)
        gamma = small.tile([64, 1], FP32, tag="gamma")
        nc.scalar.activation(gamma[:], pfc2[:], ACT.Sigmoid)
        gamma128 = small.tile([128, 1], FP32, tag="g128")
        nc.vector.tensor_copy(gamma128[0:CH, :], gamma[:])
        nc.sync.dma_start(gamma128[CH:128, :], gamma[:])
        # gamma-scaled, K=128/M=128 zero-padded att weights (per input half)
        attgA = small.tile([128, 2 * C], BF16, tag="attgA")
        attgB = small.tile([128, 2 * C], BF16, tag="attgB")
        nc.vector.memset(attgA[:, C:], 0.0)
        nc.vector.memset(attgB[:, C:], 0.0)
        nc.vector.memset(attgA[CH:128, 0:C], 0.0)
        nc.vector.memset(attgB[0:CH, 0:C], 0.0)
        nc.vector.tensor_scalar(
            attgA[0:CH, 0:C], attt[0:CH, :], gamma128[0:CH, 0:1], None, OP.mult
        )
        nc.vector.tensor_scalar(
            attgB[CH:128, 0:C], attt[CH:128, :], gamma128[CH:128, 0:1], None, OP.mult
        )
        if debug:
            nc.sync.dma_start(dbg["dbg_se"][:], st[:, 0:8])
            nc.sync.dma_start(dbg["dbg_gamma"][:], gamma[:])

        exp_gather(n_dct - 1)

        prec.release()
        smallD.release()

        if debug:
            xpd = dpool.tile([128, hh, w], FP32, tag="xpd")
            for c in range(n_dct):
                nc.vector.tensor_copy(
                    xpd[:, c * XPC : (c + 1) * XPC, :], xp_tiles[c][:]
                )
            nc.sync.dma_start(dbg["dbg_xp"][:], xpd[:])
            y1d = dpool.tile([128, hh, w], FP32, tag="y1d")
            nc.vector.tensor_copy(y1d[:], y1[:])
            nc.sync.dma_start(dbg["dbg_y1"][:], y1d[:])

        # ---------------- x_all = xp - y1 ----------------
        pxa = tc.alloc_tile_pool(name="pxa", bufs=1)
        x_all = pxa.tile([128, hh + 2, w + 2], BF16)
        nc.vector.memset(x_all[:, :, 0], 0.0)
        nc.vector.memset(x_all[:, :, w + 1], 0.0)
        nc.vector.memset(x_all[0:CH, 0, :], 0.0)
        nc.vector.memset(x_all[CH:128, hh + 1, :], 0.0)
        # pre-zero the halo rows: conv1 groups read them (x0-weighted) before
        # the halo DMAs land, and uninitialized bf16 garbage could be NaN
        nc.vector.memset(x_all[CH:128, 0, :], 0.0)
        nc.vector.memset(x_all[0:CH, hh + 1, :], 0.0)

        for c in range(n_dct):
            if c < 2:
                eng = nc.gpsimd if c == 1 else nc.vector
                eng.tensor_tensor(
                    x_all[:, 1 + c * XPC : 1 + (c + 1) * XPC, 1 : w + 1],
                    xp_tiles[c][:],
                    y1[:, c * XPC : (c + 1) * XPC, :],
                    OP.subtract,
                )
            else:
                hxc = XPC // 2
                nc.vector.tensor_tensor(
                    x_all[:, 1 + c * XPC : 1 + c * XPC + hxc, 1 : w + 1],
                    xp_tiles[c][:, 0:hxc, :],
                    y1[:, c * XPC : c * XPC + hxc, :],
                    OP.subtract,
                )
                nc.gpsimd.tensor_tensor(
                    x_all[:, 1 + c * XPC + hxc : 1 + (c + 1) * XPC, 1 : w + 1],
                    xp_tiles[c][:, hxc:XPC, :],
                    y1[:, c * XPC + hxc : (c + 1) * XPC, :],
                    OP.subtract,
                )
        pxp.release()

        ATT_G = 8
        n_att_g = hh // ATT_G
        FLAT = ATT_G * w
        AN = 512 if FLAT % 512 == 0 else 384
        n_fl = FLAT // AN
        y1v = y1[:].rearrange("p a b -> p (a b)")

        # ---------------- conv1 -> o1, interleaved with att ----------------
        po1 = tc.alloc_tile_pool(name="po1", bufs=1)
        o1 = po1.tile([C2, h + 2, w + 2], BF16)
        nc.vector.memset(o1[:, 0, :], 0.0)
        nc.vector.memset(o1[:, h + 1, :], 0.0)
        nc.vector.memset(o1[:, :, 0], 0.0)
        nc.vector.memset(o1[:, :, w + 1], 0.0)

        RT = 2
        n_c1 = hh // RT
        SSR = 16  # image rows per conv1 roll strip
        n_ss = hh // SSR
        proll = tc.alloc_tile_pool(name="proll", bufs=2)

        def att_group(hf, g):
            pb = hf * CH
            attg = attgA if hf == 0 else attgB
            xc = small.tile([128, ATT_G, w], BF16, tag="xc")
            base = g * FLAT
            xcv = xc[pb : pb + CH, :, :].rearrange("p a b -> p (a b)")
            for f in range(n_fl):
                pa = psA.tile([128, AN], FP32, tag="ps")
                nc.tensor.matmul(
                    pa[:], attg[:], y1v[:, base + f * AN : base + (f + 1) * AN],
                    start=True, stop=True,
                )
                nc.scalar.activation(
                    xcv[:, f * AN : (f + 1) * AN], pa[0:C, :], ACT.Relu,
                    bias=attbt[:, 0:1],
                )
            sl = x_all[pb : pb + CH, 1 + g * ATT_G : 1 + (g + 1) * ATT_G, 1 : w + 1]
            eng = nc.gpsimd if hf == 1 else nc.vector
            eng.tensor_tensor(sl, sl, xc[pb : pb + CH, :, :], OP.add)

        def conv1_strip(hf, ss):
            pb = hf * CH
            rollT = proll.tile([128, 17, w + 2], BF16, tag="roll")
            r0 = SSR * ss
            nc.sync.dma_start(
                rollT[0:CH, 0:16, :], x_all[pb : pb + CH, r0 : r0 + 16, :]
            )
            nc.sync.dma_start(
                rollT[CH:128, 0:17, :], x_all[pb : pb + CH, r0 + 1 : r0 + 18, :]
            )
            for gg in range(SSR // RT):
                pc = psC.tile([C2, RT * w], FP32, tag="pc")
                j = RT * gg
                for dx in range(3):
                    nc.tensor.matmul(
                        pc[:], w1pair[:, dx, :], rollT[:, j : j + 2, dx : dx + w],
                        start=(dx == 0), stop=False,
                    )
                for dx in range(3):
                    nc.tensor.matmul(
                        pc[:], w1sing[:, dx, :],
                        rollT[:, j + 1 : j + 3, dx : dx + w],
                        start=False, stop=(dx == 2),
                    )
                grow = hf * hh + r0 + j
                dst = o1[:, 1 + grow : 1 + grow + RT, 1 : w + 1]
                if gg % 2 == 0:
                    nc.scalar.activation(dst, pc[:], ACT.Relu, bias=b1t[:, 0:1])
                else:
                    nc.vector.scalar_tensor_tensor(
                        dst, pc[:], b1t[:, 0:1],
                        zerot[:, 0:1, None].to_broadcast((C2, RT, w)),
                        OP.add, OP.max,
                    )

        for g in range(n_att_g):
            att_group(0, g)
            att_group(1, g)
            if g == 0:
                # halo: half0's bottom pad row <- half1 row 1
                nc.sync.dma_start(x_all[0:CH, hh + 1, :], x_all[CH:128, 1, :])
            if g >= 2 and g % 2 == 0:
                ss = (g - 2) // 2
                conv1_strip(0, ss)
                if ss >= 1:
                    conv1_strip(1, ss)
        def conv2_group(g):
            pc = psC.tile([C2, RT * w], FP32, tag="pc")
            lr = g * RT
            for tap in range(9):
                dy, dx = divmod(tap, 3)
                rhs = o1[:, lr + dy : lr + dy + RT, dx : dx + w]
                nc.tensor.matmul(
                    pc[:], w2t[:, tap, :], rhs, start=(tap == 0), stop=(tap == 8)
                )
            stg = small.tile([C2, RT * w], FP32, tag="ostg")
            if g % 2 == 0:
                nc.scalar.activation(stg[:], pc[:], ACT.Relu, bias=b2t[:, 0:1])
            else:
                nc.vector.scalar_tensor_tensor(
                    stg[:], pc[:], b2t[:, 0:1],
                    zerot[:, 0:1].to_broadcast((C2, RT * w)),
                    OP.add, OP.max,
                )
            nc.sync.dma_start(out[:, lr : lr + RT, :], stg[:])

        # halo: half1's top pad row <- half0's last image row
        nc.sync.dma_start(x_all[CH:128, 0, :], x_all[0:CH, hh, :])
        conv1_strip(0, n_ss - 1)
        conv1_strip(1, n_ss - 1)
        conv1_strip(1, 0)
        py1.release()

        if debug:
            o1d = dpool.tile([C2, h + 2, w + 2], FP32, tag="o1d")
            nc.vector.tensor_copy(o1d[:], o1[:])
            nc.sync.dma_start(dbg["dbg_o1"][:], o1d[:])

        # ---------------- conv2 -> out ----------------
        n_c2 = h // RT
        for g in range(n_c2):
            conv2_group(g)

        proll.release()
        po1.release()
        pxa.release()

    nc.finalize()
    return nc


_NC_CACHE = {}


def _get_nc(H=384, W=384, debug=False):
    key = (H, W, debug)
    if key not in _NC_CACHE:
        _NC_CACHE[key] = build_nc(H=H, W=W, debug=debug)
    return _NC_CACHE[key]


def kernel(x, w1, b1, w2, b2, att_conv_w, att_conv_b, fc1_w, fc2_w):
    x = np.ascontiguousarray(np.asarray(x, np.float32))
    B = x.shape[0]
    nc = _get_nc(x.shape[2], x.shape[3])
    shared = {
        "w1": np.ascontiguousarray(np.asarray(w1, np.float32)),
        "b1": np.ascontiguousarray(np.asarray(b1, np.float32)),
        "w2": np.ascontiguousarray(np.asarray(w2, np.float32)),
        "b2": np.ascontiguousarray(np.asarray(b2, np.float32)),
        "att_conv_w": np.ascontiguousarray(np.asarray(att_conv_w, np.float32)),
        "att_conv_b": np.ascontiguousarray(np.asarray(att_conv_b, np.float32)),
        "fc1_w": np.ascontiguousarray(np.asarray(fc1_w, np.float32)),
        "fc2_w": np.ascontiguousarray(np.asarray(fc2_w, np.float32)),
    }
    in_maps = [dict(shared, x=np.ascontiguousarray(x[i])) for i in range(B)]
    res = run_bass_kernel_spmd(nc, in_maps, core_ids=list(range(B)))
    return np.stack([res.results[i]["out"] for i in range(B)], axis=0)

